# revision 23
# baseline (speedup 1.0000x reference)
"""Trainium2 Bass kernel for the DWA middle layer (moe_routing).

Math (factored form of the reference; W_assembled is never materialized):
    t     = h_A @ V_flat^T                      # [B, N*R]
    s     = t * repeat(alpha, R, axis=1)        # [B, N*R]
    h_T   = s @ U_flat^T + h_A @ W_base^T + [alpha, 1] @ [bias_pool; b_base]
    out   = LayerNorm(h_A + gamma * h_T) * ln_scale + ln_bias

Sharding: data-parallel over the batch dim (32 rows per core, 8 cores);
weights replicated.  The kernel is HBM-bound on the three 1024x1024
weight streams, so they are sent as fp8 e4m3 (scaled x32 on the host;
the scale is folded into alpha, the bias matrix and gamma, so the
device math is unchanged up to fp8 rounding — measured end-to-end
rel-err ~2e-3 against the fp32 reference, well inside the 2e-2 gate).
Matmuls run in DoubleRow fp8 mode (256-deep contraction per pass,
~512 PE cycles per [256k x 32m x 512n] instruction).

Perf notes (from perfetto/NTFF analysis of earlier revisions):
  - A stream of dummy fp8 matmuls at kernel start keeps the PE HAM
    activity window busy so real matmuls run at 2.4 GHz, not the
    1.2 GHz cold clock.
  - The NEFF exit protocol drains every allocated DMA queue ring
    (~115ns x 16 rings per issuing engine), so ALL loads ride one
    HWDGE ring (sync) — the SDMA engines already round-robin between
    queues, so a second ring adds no aggregate bandwidth, only tail.
  - Per-DMA fixed overhead is ~1.3us, so small inputs are packed into
    few blob DMAs and h_A^T(fp8) is concatenated with the V matrix.
  - Host-side prep only re-lays-out / scales / casts inputs; all
    arithmetic between tensors runs on device.
"""

import os
from contextlib import ExitStack

import ml_dtypes
import numpy as np

import concourse.bacc as bacc
import concourse.mybir as mybir
import concourse.tile as tile
from concourse import bass_utils, masks

F32 = mybir.dt.float32
BF16 = mybir.dt.bfloat16
F8 = mybir.dt.float8e4
NP_F8 = ml_dtypes.float8_e4m3
NP_BF16 = ml_dtypes.bfloat16

D = 1024          # d_A == d_B
B_CORE = 32       # batch rows per core
N_EXP = 64        # experts
R_RANK = 16       # rank per expert
N_CORES = 8
KT = 8            # 128-deep contraction tiles
JT = 4            # DoubleRow 256-deep contraction tiles
NH = 2            # output halves of 512
WSC = 32.0        # fp8 weight scale (folded into alpha/bias/gamma)
XW = JT * 2 * B_CORE  # 256 columns of h_A^T tiles
N_WU = int(os.environ.get("DWA_WARMUP_MM", "13"))  # PE warm-up matmuls

_COMPILED = {}


def _build(general_ln):
    nc = bacc.Bacc("TRN2", debug=False, num_devices=N_CORES,
                   enable_partition_id=False)

    # [128, 256 (h_A^T) | 1056 (bias pad) | 64 (alpha/4 pad) | 8192 (V)]
    xv_d = nc.dram_tensor("xv", [128, XW + D + B_CORE + N_EXP + KT * D], F8,
                          kind="ExternalInput")
    wt_d = nc.dram_tensor("wt", [128, KT * D], F8, kind="ExternalInput")
    ut_d = nc.dram_tensor("ut", [128, KT * D], F8, kind="ExternalInput")
    # [32, 1024 (h_A) | 1 (gamma/32)]
    fb_d = nc.dram_tensor("fb", [B_CORE, D + 1], F32, kind="ExternalInput")

    if general_ln:
        lns_d = nc.dram_tensor("lns", [1, D], F32, kind="ExternalInput")
        lnb_d = nc.dram_tensor("lnb", [1, D], F32, kind="ExternalInput")
    else:
        lns_d = lnb_d = None
    out_d = nc.dram_tensor("out", [B_CORE, D], F32, kind="ExternalOutput")

    with ExitStack() as ctx:
        tc = ctx.enter_context(tile.TileContext(nc))
        _emit(ctx, tc, general_ln, xv_d, wt_d, ut_d, fb_d,
              lns_d, lnb_d, out_d)

    nc.compile()
    return nc


def _emit(ctx, tc, general_ln, xv_d, wt_d, ut_d, fb_d,
          lns_d, lnb_d, out_d):
    nc = tc.nc
    MULT = mybir.AluOpType.mult
    ADD = mybir.AluOpType.add
    SUB = mybir.AluOpType.subtract
    SQRT = mybir.ActivationFunctionType.Sqrt
    DR = mybir.MatmulPerfMode.DoubleRow

    wpool = ctx.enter_context(tc.tile_pool(name="weights", bufs=1))
    sm = ctx.enter_context(tc.tile_pool(name="small", bufs=1))
    pp = ctx.enter_context(tc.tile_pool(name="psum", bufs=1, space="PSUM"))

    BBW = D + B_CORE
    HDR = XW + BBW + N_EXP
    HH = JT * D
    xv_sb = wpool.tile([128, HDR + KT * D], F8, tag="xv")
    wt_sb = wpool.tile([128, KT * D], F8, tag="wt")
    ut_sb = wpool.tile([128, KT * D], F8, tag="ut")
    x8_sb = xv_sb[:, :XW]
    bb_sb = xv_sb[:, XW:XW + BBW]
    al_sb = xv_sb[:, XW + BBW:HDR]
    vw = [(xv_sb[:, HDR:HDR + HH], wt_sb[:, :HH]),
          (xv_sb[:, HDR + HH:], wt_sb[:, HH:])]

    fb_sb = sm.tile([B_CORE, D + 1], F32, tag="fb")
    ha_sb = fb_sb[:, :D]
    gmc_sb = fb_sb[:, D:D + 1]
    bp_sb = bb_sb[:N_EXP + 1, :D]
    alt_sb = bb_sb[:N_EXP + 1, D:D + B_CORE]
    wu_sb = sm.tile([128, 2 * B_CORE + 1024], F8, tag="wu")
    ident = sm.tile([B_CORE, B_CORE], F32, tag="ident")
    s_sb = sm.tile([B_CORE, D], F32, tag="s")
    st_sb = sm.tile([128, KT * B_CORE], F8, tag="st")    # s^T tiles
    hpre_sb = sm.tile([B_CORE, D], F32, tag="hpre")
    sq_sb = sm.tile([B_CORE, D], F32, tag="sq")
    out_sb = sm.tile([B_CORE, D], F32, tag="out")
    sum_h = [sm.tile([B_CORE, 1], F32, tag=f"sumh{h}", name=f"sumh{h}")
             for h in range(NH)]
    sum_q = sm.tile([B_CORE, 1], F32, tag="sumq")
    ssq_a = sm.tile([B_CORE, 1], F32, tag="ssqa")
    ssq_b = sm.tile([B_CORE, 1], F32, tag="ssqb")
    ssq_c2 = sm.tile([B_CORE, 1], F32, tag="ssqc2")
    m_c = sm.tile([B_CORE, 1], F32, tag="mc")
    ssqs_c = sm.tile([B_CORE, 1], F32, tag="ssqsc")
    msq_c = sm.tile([B_CORE, 1], F32, tag="msqc")
    var_c = sm.tile([B_CORE, 1], F32, tag="varc")
    std_c = sm.tile([B_CORE, 1], F32, tag="stdc")
    istd_c = sm.tile([B_CORE, 1], F32, tag="istdc")
    nmi_c = sm.tile([B_CORE, 1], F32, tag="nmic")
    eps_c = sm.tile([B_CORE, 1], F32, tag="epsc")
    warm_c = sm.tile([B_CORE, 1], F32, tag="warmc")
    if general_ln:
        lnsr_sb = sm.tile([B_CORE, D], F32, tag="lnsr")
        lnbr_sb = sm.tile([B_CORE, D], F32, tag="lnbr")
        y_sb = sm.tile([B_CORE, D], F32, tag="y")
        t2_sb = sm.tile([B_CORE, D], F32, tag="t2")

    # ---- one DMA per engine queue: queue completions are serialized
    # within a ring (~2us each) but run in parallel across rings, and
    # every engine owns a hardware DMA queue.  xv halves go on the two
    # HWDGE rings; W and U stream on the vector/tensor queues; fb on
    # the gpsimd SWDGE queue. ----
    C1 = HDR + HH
    nc.scalar.dma_start(out=fb_sb[:], in_=fb_d.ap())
    nc.sync.dma_start(out=xv_sb[:, :HDR], in_=xv_d.ap()[:, :HDR])
    nc.sync.dma_start(out=xv_sb[:, HDR:C1], in_=xv_d.ap()[:, HDR:C1])
    nc.scalar.dma_start(out=xv_sb[:, C1:], in_=xv_d.ap()[:, C1:])
    nc.sync.dma_start(out=wt_sb[:], in_=wt_d.ap())
    nc.scalar.dma_start(out=ut_sb[:], in_=ut_d.ap())
    if general_ln:
        nc.sync.dma_start(out=lnsr_sb[:],
                          in_=lns_d.ap().broadcast_to([B_CORE, D]))
        nc.scalar.dma_start(out=lnbr_sb[:],
                          in_=lnb_d.ap().broadcast_to([B_CORE, D]))

    nc.vector.memset(eps_c[:], 1e-5)
    nc.vector.memset(wu_sb[:], 0.25)
    masks.make_identity(nc, ident[:])
    # preload both ACT tables (Square, Sqrt) off the critical path
    nc.scalar.activation(warm_c[:], eps_c[:],
                         mybir.ActivationFunctionType.Square)
    nc.scalar.activation(warm_c[:], eps_c[:], SQRT, bias=eps_c[:], scale=1.0)

    def dr_view(ap):
        return ap.rearrange("p (two n) -> p two n", two=2)

    def dr_rhs(w_sb, h, j):
        # w_sb is a [128, 4096] half-block: j-tiles of 1024 columns
        return dr_view(w_sb[:, j * D:(j + 1) * D])

    def dr_lhs(x_sb, j):
        off = j * 2 * B_CORE
        return dr_view(x_sb[:, off:off + 2 * B_CORE])

    t_ps = [pp.tile([B_CORE, 512], F32, tag=f"t{h}", name=f"t_ps{h}")
            for h in range(NH)]
    h_ps = [pp.tile([B_CORE, 512], F32, tag=f"h{h}", name=f"h_ps{h}")
            for h in range(NH)]
    tr_ps = [pp.tile([128, 128], F32, tag=f"tr{h}", name=f"tr_ps{h}")
             for h in range(NH)]
    wu_ps = pp.tile([B_CORE, 512], F32, tag="wu", name="wu_ps")

    # ---- PE warm-up: keep the HAM activity window busy while the
    # first weight chunks stream in, so real matmuls run at 2.4 GHz ----
    wu_lhs = dr_view(wu_sb[:, :2 * B_CORE])
    wu_rhs = dr_view(wu_sb[:, 2 * B_CORE:])
    for i in range(N_WU):
        nc.tensor.matmul(wu_ps[:], wu_lhs, wu_rhs,
                         start=True, stop=True, perf_mode=DR)

    # ---- t = h_A @ V^T ; s = t * repeat(alpha/32, R); s^T tiles ----
    ctx.enter_context(tc.high_priority())
    for h in range(NH):
        for j in range(JT):
            nc.tensor.matmul(t_ps[h][:], dr_lhs(x8_sb, j), dr_rhs(vw[h][0], h, j),
                             start=(j == 0), stop=(j == JT - 1), perf_mode=DR)
        o3 = s_sb[:, 512 * h:512 * (h + 1)].rearrange(
            "p (n r) -> p n r", r=R_RANK)
        i3 = t_ps[h][:].rearrange("p (n r) -> p n r", r=R_RANK)
        a3 = al_sb[:B_CORE, 32 * h:32 * (h + 1)].unsqueeze(-1).broadcast_to(
            [B_CORE, 32, R_RANK])
        nc.vector.scalar_tensor_tensor(
            out=o3, in0=i3, scalar=0.125, in1=a3, op0=MULT, op1=MULT)
        # four transposes into one PSUM bank, one fp8-casting copy out
        for kk in range(4):
            k = 4 * h + kk
            nc.tensor.transpose(tr_ps[h][:, 32 * kk:32 * (kk + 1)],
                                s_sb[:, 128 * k:128 * (k + 1)], ident[:])
        nc.vector.tensor_copy(st_sb[:, 128 * h:128 * (h + 1)], tr_ps[h][:])

    # ---- 32*h_T = [al,1]@bp' + h_A @ (32W)^T + s @ (32U)^T ----
    # U arrives last, so U matmuls close each accumulation group.
    tc.cur_priority = 10**6
    for h in range(NH):
        nc.tensor.matmul(h_ps[h][:], alt_sb[:],
                         bp_sb[:, 512 * h:512 * (h + 1)],
                         start=True, stop=False)
    for h in range(NH):
        for j in range(JT):
            nc.tensor.matmul(h_ps[h][:], dr_lhs(x8_sb, j), dr_rhs(vw[h][1], h, j),
                             start=False, stop=False, perf_mode=DR)
    for h in range(NH):
        for j in range(JT):
            nc.tensor.matmul(h_ps[h][:], dr_lhs(st_sb, j),
                             dr_view(ut_sb[:, h * HH + j * D:
                                           h * HH + (j + 1) * D]),
                             start=False, stop=(j == JT - 1), perf_mode=DR)
        if h == 0:
            sl = slice(0, 512)
            # h_pre = (gamma/32)*(32 h_T) + h_A, with row-sums for the mean
            nc.vector.scalar_tensor_tensor(
                out=hpre_sb[:, sl], in0=h_ps[0][:], scalar=gmc_sb,
                in1=ha_sb[:, sl], op0=MULT, op1=ADD,
                accum_out=sum_h[0][:])
            nc.scalar.activation(sq_sb[:, sl], hpre_sb[:, sl],
                                 mybir.ActivationFunctionType.Square,
                                 accum_out=ssq_a[:])
        else:
            # critical-path half: square split across ACT and DVE
            sl = slice(512, 1024)
            nc.vector.scalar_tensor_tensor(
                out=hpre_sb[:, sl], in0=h_ps[1][:], scalar=gmc_sb,
                in1=ha_sb[:, sl], op0=MULT, op1=ADD,
                accum_out=sum_h[1][:])
            nc.scalar.activation(sq_sb[:, 512:768], hpre_sb[:, 512:768],
                                 mybir.ActivationFunctionType.Square,
                                 accum_out=ssq_b[:])
            nc.vector.scalar_tensor_tensor(
                out=sq_sb[:, 768:1024], in0=hpre_sb[:, 768:1024], scalar=1.0,
                in1=hpre_sb[:, 768:1024], op0=MULT, op1=MULT,
                accum_out=ssq_c2[:])

    # ---- LayerNorm via E[x^2] - E[x]^2 ----
    # m_c holds D*mean; the 1/D folds into msq and nmi scalars
    nc.vector.tensor_add(m_c[:], sum_h[0][:], sum_h[1][:])
    nc.vector.tensor_scalar(out=ssqs_c[:], in0=ssq_a[:], scalar1=ssq_b[:],
                            scalar2=ssq_c2[:], op0=ADD, op1=ADD)
    nc.vector.tensor_scalar(out=msq_c[:], in0=m_c[:], scalar1=m_c[:],
                            scalar2=1.0 / (D * D), op0=MULT, op1=MULT)
    nc.vector.scalar_tensor_tensor(
        out=var_c[:], in0=ssqs_c[:], scalar=1.0 / D, in1=msq_c[:],
        op0=MULT, op1=SUB)
    nc.scalar.activation(std_c[:], var_c[:], SQRT, bias=eps_c[:], scale=1.0)
    nc.vector.reciprocal(istd_c[:], std_c[:])
    nc.vector.tensor_scalar(out=nmi_c[:], in0=m_c[:], scalar1=istd_c[:],
                            scalar2=-1.0 / D, op0=MULT, op1=MULT)

    for h in range(NH):
        sl = slice(512 * h, 512 * (h + 1))
        if general_ln:
            # out = hpre*istd*lns + (lnb - m*istd*lns)
            nc.vector.scalar_tensor_tensor(
                out=t2_sb[:, sl], in0=lnsr_sb[:, sl], scalar=nmi_c[:],
                in1=lnbr_sb[:, sl], op0=MULT, op1=ADD)
            nc.vector.scalar_tensor_tensor(
                out=y_sb[:, sl], in0=hpre_sb[:, sl], scalar=istd_c[:],
                in1=lnsr_sb[:, sl], op0=MULT, op1=MULT)
            nc.vector.tensor_add(out_sb[:, sl], y_sb[:, sl], t2_sb[:, sl])
        else:
            # ln_scale==1, ln_bias==0: out = hpre*istd - m*istd
            if h == 0:
                nc.scalar.activation(
                    out_sb[:, sl], hpre_sb[:, sl],
                    mybir.ActivationFunctionType.Identity,
                    scale=istd_c[:], bias=nmi_c[:])
            else:
                nc.vector.tensor_scalar(
                    out=out_sb[:, sl], in0=hpre_sb[:, sl],
                    scalar1=istd_c[:], scalar2=nmi_c[:], op0=MULT, op1=ADD)
        eng = nc.sync if h == 0 else nc.scalar
        eng.dma_start(out=out_d.ap()[:, sl], in_=out_sb[:, sl])


def _dr_layout(m, scale):
    """[1024 k, 1024 out] f32 -> [128, (h j i n)] fp8 DoubleRow layout."""
    a = np.asarray(m * scale, dtype=NP_F8)
    # k -> (j, i, p), out -> (h, n); final [p, h, j, i, n]
    a = a.reshape(JT, 2, 128, NH, 512).transpose(2, 3, 0, 1, 4)
    return np.ascontiguousarray(a.reshape(128, KT * D))


def _prep_in_maps(inputs, general_ln):
    def f32c(x):
        return np.ascontiguousarray(np.asarray(x, dtype=np.float32))

    h_a = f32c(inputs["h_A"])
    alpha = f32c(inputs["alpha"])
    pool = np.asarray(inputs["pool_vectors"], dtype=np.float32)
    w_base = np.asarray(inputs["W_base"], dtype=np.float32)

    # pool_vectors rows: [U_n (D*R) | V_n (R*D) | bias_n (D)]
    u = pool[:, :D * R_RANK].reshape(N_EXP, D, R_RANK)
    v = pool[:, D * R_RANK:2 * D * R_RANK].reshape(N_EXP, R_RANK, D)
    bias_pool = pool[:, 2 * D * R_RANK:]                    # [64, D]
    bb = np.asarray(inputs["b_base"], dtype=np.float32).reshape(1, D)
    # fp8 weights are scaled x32; alpha carries 1/32, so the bias rows
    # need x(32*32) for the pool part and x32 for b_base
    bp = np.concatenate([bias_pool * (4 * WSC), bb * (4 * WSC)], axis=0)
    vt = _dr_layout(v.reshape(N_EXP * R_RANK, D).T, WSC)   # [a, (n,r)]
    wt = _dr_layout(w_base.T, WSC)                          # [a, c]
    ut = _dr_layout(u.transpose(0, 2, 1).reshape(N_EXP * R_RANK, D), WSC)
    gm = float(np.asarray(inputs["gamma"], dtype=np.float32)) / WSC

    in_maps = []
    for k in range(N_CORES):
        rows = slice(B_CORE * k, B_CORE * (k + 1))
        xt = h_a[rows].T                                    # [1024, 32]
        x8 = np.asarray(xt.reshape(JT, 2, 128, B_CORE).transpose(2, 0, 1, 3)
                        .reshape(128, XW), dtype=NP_F8)
        fb = np.concatenate(
            [h_a[rows], np.full((B_CORE, 1), gm, np.float32)], axis=1)
        alt = np.concatenate(
            [alpha[rows] / 4.0, np.full((B_CORE, 1), 0.25, np.float32)],
            axis=1).T
        bbb = np.concatenate([bp, alt], axis=1)             # [65, 1056]
        bb8 = np.zeros((128, D + B_CORE), np.float32)
        bb8[:N_EXP + 1] = bbb
        al8 = np.zeros((128, N_EXP), np.float32)
        al8[:B_CORE] = alpha[rows] / 4.0
        xvb = np.concatenate(
            [x8, np.asarray(bb8, dtype=NP_F8), np.asarray(al8, dtype=NP_F8),
             vt], axis=1)
        im = {
            "xv": np.ascontiguousarray(xvb),
            "fb": f32c(fb), "wt": wt, "ut": ut,
        }
        if general_ln:
            im["lns"] = f32c(inputs["ln_scale"]).reshape(1, D)
            im["lnb"] = f32c(inputs["ln_bias"]).reshape(1, D)
        in_maps.append(im)
    return in_maps


def _is_general_ln(inputs):
    lns = np.asarray(inputs["ln_scale"], dtype=np.float32)
    lnb = np.asarray(inputs["ln_bias"], dtype=np.float32)
    return not (np.all(lns == 1.0) and np.all(lnb == 0.0))


def get_compiled(general_ln=False):
    key = bool(general_ln)
    if key not in _COMPILED:
        _COMPILED[key] = _build(key)
    return _COMPILED[key]


def kernel(**inputs):
    general_ln = _is_general_ln(inputs)
    nc = get_compiled(general_ln)
    in_maps = _prep_in_maps(inputs, general_ln)
    res = bass_utils.run_bass_kernel_spmd(
        nc, in_maps, core_ids=list(range(N_CORES)))
    return np.concatenate([r["out"] for r in res.results], axis=0)


# revision 24
# speedup vs baseline: 1.0273x; 1.0273x over previous
"""Trainium2 Bass kernel for the DWA middle layer (moe_routing).

Math (factored form of the reference; W_assembled is never materialized):
    t     = h_A @ V_flat^T                      # [B, N*R]
    s     = t * repeat(alpha, R, axis=1)        # [B, N*R]
    h_T   = s @ U_flat^T + h_A @ W_base^T + [alpha, 1] @ [bias_pool; b_base]
    out   = LayerNorm(h_A + gamma * h_T) * ln_scale + ln_bias

Sharding: data-parallel over the batch dim (32 rows per core, 8 cores);
weights replicated.  The kernel is HBM-bound on the three 1024x1024
weight streams, so they are sent as fp8 e4m3 (scaled x32 on the host;
the scale is folded into alpha, the bias matrix and gamma, so the
device math is unchanged up to fp8 rounding — measured end-to-end
rel-err ~2e-3 against the fp32 reference, well inside the 2e-2 gate).
Matmuls run in DoubleRow fp8 mode (256-deep contraction per pass,
~512 PE cycles per [256k x 32m x 512n] instruction).

Perf notes (from perfetto/NTFF analysis of earlier revisions):
  - A stream of dummy fp8 matmuls at kernel start keeps the PE HAM
    activity window busy so real matmuls run at 2.4 GHz, not the
    1.2 GHz cold clock.
  - The NEFF exit protocol drains every allocated DMA queue ring
    (~115ns x 16 rings per issuing engine), so ALL loads ride one
    HWDGE ring (sync) — the SDMA engines already round-robin between
    queues, so a second ring adds no aggregate bandwidth, only tail.
  - Per-DMA fixed overhead is ~1.3us, so small inputs are packed into
    few blob DMAs and h_A^T(fp8) is concatenated with the V matrix.
  - Host-side prep only re-lays-out / scales / casts inputs; all
    arithmetic between tensors runs on device.
"""

import os
from contextlib import ExitStack

import ml_dtypes
import numpy as np

import concourse.bacc as bacc
import concourse.mybir as mybir
import concourse.tile as tile
from concourse import bass_utils, masks

F32 = mybir.dt.float32
BF16 = mybir.dt.bfloat16
F8 = mybir.dt.float8e4
NP_F8 = ml_dtypes.float8_e4m3
NP_BF16 = ml_dtypes.bfloat16

D = 1024          # d_A == d_B
B_CORE = 32       # batch rows per core
N_EXP = 64        # experts
R_RANK = 16       # rank per expert
N_CORES = 8
KT = 8            # 128-deep contraction tiles
JT = 4            # DoubleRow 256-deep contraction tiles
NH = 2            # output halves of 512
WSC = 32.0        # fp8 weight scale (folded into alpha/bias/gamma)
XW = JT * 2 * B_CORE  # 256 columns of h_A^T tiles
N_WU = int(os.environ.get("DWA_WARMUP_MM", "13"))  # PE warm-up matmuls

_COMPILED = {}


def _build(general_ln):
    nc = bacc.Bacc("TRN2", debug=False, num_devices=N_CORES,
                   enable_partition_id=False)

    # [128, 256 (h_A^T) | 1056 (bias pad) | 64 (alpha/4 pad) | 8192 (V)]
    xv_d = nc.dram_tensor("xv", [128, XW + D + B_CORE + N_EXP + KT * D], F8,
                          kind="ExternalInput")
    wt_d = nc.dram_tensor("wt", [128, KT * D], F8, kind="ExternalInput")
    ut_d = nc.dram_tensor("ut", [128, KT * D], F8, kind="ExternalInput")
    # [32, 1024 (h_A) | 1 (gamma/32)]
    fb_d = nc.dram_tensor("fb", [B_CORE, D + 1], F32, kind="ExternalInput")

    if general_ln:
        lns_d = nc.dram_tensor("lns", [1, D], F32, kind="ExternalInput")
        lnb_d = nc.dram_tensor("lnb", [1, D], F32, kind="ExternalInput")
    else:
        lns_d = lnb_d = None
    out_d = nc.dram_tensor("out", [B_CORE, D], F32, kind="ExternalOutput")

    with ExitStack() as ctx:
        tc = ctx.enter_context(tile.TileContext(nc))
        _emit(ctx, tc, general_ln, xv_d, wt_d, ut_d, fb_d,
              lns_d, lnb_d, out_d)

    nc.compile()
    return nc


def _emit(ctx, tc, general_ln, xv_d, wt_d, ut_d, fb_d,
          lns_d, lnb_d, out_d):
    nc = tc.nc
    MULT = mybir.AluOpType.mult
    ADD = mybir.AluOpType.add
    SUB = mybir.AluOpType.subtract
    SQRT = mybir.ActivationFunctionType.Sqrt
    DR = mybir.MatmulPerfMode.DoubleRow

    wpool = ctx.enter_context(tc.tile_pool(name="weights", bufs=1))
    sm = ctx.enter_context(tc.tile_pool(name="small", bufs=1))
    pp = ctx.enter_context(tc.tile_pool(name="psum", bufs=1, space="PSUM"))

    BBW = D + B_CORE
    HDR = XW + BBW + N_EXP
    HH = JT * D
    xv_sb = wpool.tile([128, HDR + KT * D], F8, tag="xv")
    wt_sb = wpool.tile([128, KT * D], F8, tag="wt")
    ut_sb = wpool.tile([128, KT * D], F8, tag="ut")
    x8_sb = xv_sb[:, :XW]
    bb_sb = xv_sb[:, XW:XW + BBW]
    al_sb = xv_sb[:, XW + BBW:HDR]
    vw = [(xv_sb[:, HDR:HDR + HH], wt_sb[:, :HH]),
          (xv_sb[:, HDR + HH:], wt_sb[:, HH:])]

    fb_sb = sm.tile([B_CORE, D + 1], F32, tag="fb")
    ha_sb = fb_sb[:, :D]
    gmc_sb = fb_sb[:, D:D + 1]
    bp_sb = bb_sb[:N_EXP + 1, :D]
    alt_sb = bb_sb[:N_EXP + 1, D:D + B_CORE]
    wu_sb = sm.tile([128, 2 * B_CORE + 1024], F8, tag="wu")
    ident = sm.tile([B_CORE, B_CORE], F32, tag="ident")
    s_sb = sm.tile([B_CORE, D], F32, tag="s")
    st_sb = sm.tile([128, KT * B_CORE], F8, tag="st")    # s^T tiles
    hpre_sb = sm.tile([B_CORE, D], F32, tag="hpre")
    sq_sb = sm.tile([B_CORE, D], F32, tag="sq")
    out_sb = sm.tile([B_CORE, D], F32, tag="out")
    sum_h = [sm.tile([B_CORE, 1], F32, tag=f"sumh{h}", name=f"sumh{h}")
             for h in range(NH)]
    sum_q = sm.tile([B_CORE, 1], F32, tag="sumq")
    ssq_a = sm.tile([B_CORE, 1], F32, tag="ssqa")
    ssq_b = sm.tile([B_CORE, 1], F32, tag="ssqb")
    ssq_c2 = sm.tile([B_CORE, 1], F32, tag="ssqc2")
    m_c = sm.tile([B_CORE, 1], F32, tag="mc")
    ssqs_c = sm.tile([B_CORE, 1], F32, tag="ssqsc")
    msq_c = sm.tile([B_CORE, 1], F32, tag="msqc")
    var_c = sm.tile([B_CORE, 1], F32, tag="varc")
    std_c = sm.tile([B_CORE, 1], F32, tag="stdc")
    istd_c = sm.tile([B_CORE, 1], F32, tag="istdc")
    nmi_c = sm.tile([B_CORE, 1], F32, tag="nmic")
    eps_c = sm.tile([B_CORE, 1], F32, tag="epsc")
    warm_c = sm.tile([B_CORE, 1], F32, tag="warmc")
    if general_ln:
        lnsr_sb = sm.tile([B_CORE, D], F32, tag="lnsr")
        lnbr_sb = sm.tile([B_CORE, D], F32, tag="lnbr")
        y_sb = sm.tile([B_CORE, D], F32, tag="y")
        t2_sb = sm.tile([B_CORE, D], F32, tag="t2")

    # ---- one DMA per engine queue: queue completions are serialized
    # within a ring (~2us each) but run in parallel across rings, and
    # every engine owns a hardware DMA queue.  xv halves go on the two
    # HWDGE rings; W and U stream on the vector/tensor queues; fb on
    # the gpsimd SWDGE queue. ----
    C1 = HDR + HH
    nc.sync.dma_start(out=xv_sb[:, :C1], in_=xv_d.ap()[:, :C1])
    nc.scalar.dma_start(out=xv_sb[:, C1:], in_=xv_d.ap()[:, C1:])
    nc.sync.dma_start(out=wt_sb[:], in_=wt_d.ap())
    nc.scalar.dma_start(out=ut_sb[:], in_=ut_d.ap())
    nc.gpsimd.dma_start(out=fb_sb[:], in_=fb_d.ap())
    if general_ln:
        nc.sync.dma_start(out=lnsr_sb[:],
                          in_=lns_d.ap().broadcast_to([B_CORE, D]))
        nc.scalar.dma_start(out=lnbr_sb[:],
                          in_=lnb_d.ap().broadcast_to([B_CORE, D]))

    nc.vector.memset(eps_c[:], 1e-5)
    nc.vector.memset(wu_sb[:], 0.25)
    masks.make_identity(nc, ident[:])
    # preload both ACT tables (Square, Sqrt) off the critical path
    nc.scalar.activation(warm_c[:], eps_c[:],
                         mybir.ActivationFunctionType.Square)
    nc.scalar.activation(warm_c[:], eps_c[:], SQRT, bias=eps_c[:], scale=1.0)

    def dr_view(ap):
        return ap.rearrange("p (two n) -> p two n", two=2)

    def dr_rhs(w_sb, h, j):
        # w_sb is a [128, 4096] half-block: j-tiles of 1024 columns
        return dr_view(w_sb[:, j * D:(j + 1) * D])

    def dr_lhs(x_sb, j):
        off = j * 2 * B_CORE
        return dr_view(x_sb[:, off:off + 2 * B_CORE])

    t_ps = [pp.tile([B_CORE, 512], F32, tag=f"t{h}", name=f"t_ps{h}")
            for h in range(NH)]
    h_ps = [pp.tile([B_CORE, 512], F32, tag=f"h{h}", name=f"h_ps{h}")
            for h in range(NH)]
    tr_ps = [pp.tile([128, 128], F32, tag=f"tr{h}", name=f"tr_ps{h}")
             for h in range(NH)]
    wu_ps = pp.tile([B_CORE, 512], F32, tag="wu", name="wu_ps")

    # ---- PE warm-up: keep the HAM activity window busy while the
    # first weight chunks stream in, so real matmuls run at 2.4 GHz ----
    wu_lhs = dr_view(wu_sb[:, :2 * B_CORE])
    wu_rhs = dr_view(wu_sb[:, 2 * B_CORE:])
    for i in range(N_WU):
        nc.tensor.matmul(wu_ps[:], wu_lhs, wu_rhs,
                         start=True, stop=True, perf_mode=DR)

    # ---- t = h_A @ V^T ; s = t * repeat(alpha/32, R); s^T tiles ----
    ctx.enter_context(tc.high_priority())
    for h in range(NH):
        for j in range(JT):
            nc.tensor.matmul(t_ps[h][:], dr_lhs(x8_sb, j), dr_rhs(vw[h][0], h, j),
                             start=(j == 0), stop=(j == JT - 1), perf_mode=DR)
        o3 = s_sb[:, 512 * h:512 * (h + 1)].rearrange(
            "p (n r) -> p n r", r=R_RANK)
        i3 = t_ps[h][:].rearrange("p (n r) -> p n r", r=R_RANK)
        a3 = al_sb[:B_CORE, 32 * h:32 * (h + 1)].unsqueeze(-1).broadcast_to(
            [B_CORE, 32, R_RANK])
        nc.vector.scalar_tensor_tensor(
            out=o3, in0=i3, scalar=0.125, in1=a3, op0=MULT, op1=MULT)
        # four transposes into one PSUM bank, one fp8-casting copy out
        for kk in range(4):
            k = 4 * h + kk
            nc.tensor.transpose(tr_ps[h][:, 32 * kk:32 * (kk + 1)],
                                s_sb[:, 128 * k:128 * (k + 1)], ident[:])
        nc.vector.tensor_copy(st_sb[:, 128 * h:128 * (h + 1)], tr_ps[h][:])

    # ---- 32*h_T = [al,1]@bp' + h_A @ (32W)^T + s @ (32U)^T ----
    # U arrives last, so U matmuls close each accumulation group.
    tc.cur_priority = 10**6
    for h in range(NH):
        nc.tensor.matmul(h_ps[h][:], alt_sb[:],
                         bp_sb[:, 512 * h:512 * (h + 1)],
                         start=True, stop=False)
    for h in range(NH):
        for j in range(JT):
            nc.tensor.matmul(h_ps[h][:], dr_lhs(x8_sb, j), dr_rhs(vw[h][1], h, j),
                             start=False, stop=False, perf_mode=DR)
    for h in range(NH):
        for j in range(JT):
            nc.tensor.matmul(h_ps[h][:], dr_lhs(st_sb, j),
                             dr_view(ut_sb[:, h * HH + j * D:
                                           h * HH + (j + 1) * D]),
                             start=False, stop=(j == JT - 1), perf_mode=DR)
        if h == 0:
            sl = slice(0, 512)
            # h_pre = (gamma/32)*(32 h_T) + h_A, with row-sums for the mean
            nc.vector.scalar_tensor_tensor(
                out=hpre_sb[:, sl], in0=h_ps[0][:], scalar=gmc_sb,
                in1=ha_sb[:, sl], op0=MULT, op1=ADD,
                accum_out=sum_h[0][:])
            nc.scalar.activation(sq_sb[:, sl], hpre_sb[:, sl],
                                 mybir.ActivationFunctionType.Square,
                                 accum_out=ssq_a[:])
        else:
            # critical-path half: square split across ACT and DVE
            sl = slice(512, 1024)
            nc.vector.scalar_tensor_tensor(
                out=hpre_sb[:, sl], in0=h_ps[1][:], scalar=gmc_sb,
                in1=ha_sb[:, sl], op0=MULT, op1=ADD,
                accum_out=sum_h[1][:])
            nc.scalar.activation(sq_sb[:, 512:768], hpre_sb[:, 512:768],
                                 mybir.ActivationFunctionType.Square,
                                 accum_out=ssq_b[:])
            nc.vector.scalar_tensor_tensor(
                out=sq_sb[:, 768:1024], in0=hpre_sb[:, 768:1024], scalar=1.0,
                in1=hpre_sb[:, 768:1024], op0=MULT, op1=MULT,
                accum_out=ssq_c2[:])

    # ---- LayerNorm via E[x^2] - E[x]^2 ----
    # m_c holds D*mean; the 1/D folds into msq and nmi scalars
    nc.vector.tensor_add(m_c[:], sum_h[0][:], sum_h[1][:])
    nc.vector.tensor_scalar(out=ssqs_c[:], in0=ssq_a[:], scalar1=ssq_b[:],
                            scalar2=ssq_c2[:], op0=ADD, op1=ADD)
    nc.vector.tensor_scalar(out=msq_c[:], in0=m_c[:], scalar1=m_c[:],
                            scalar2=1.0 / (D * D), op0=MULT, op1=MULT)
    nc.vector.scalar_tensor_tensor(
        out=var_c[:], in0=ssqs_c[:], scalar=1.0 / D, in1=msq_c[:],
        op0=MULT, op1=SUB)
    nc.scalar.activation(std_c[:], var_c[:], SQRT, bias=eps_c[:], scale=1.0)
    nc.vector.reciprocal(istd_c[:], std_c[:])
    nc.vector.tensor_scalar(out=nmi_c[:], in0=m_c[:], scalar1=istd_c[:],
                            scalar2=-1.0 / D, op0=MULT, op1=MULT)

    for h in range(NH):
        sl = slice(512 * h, 512 * (h + 1))
        if general_ln:
            # out = hpre*istd*lns + (lnb - m*istd*lns)
            nc.vector.scalar_tensor_tensor(
                out=t2_sb[:, sl], in0=lnsr_sb[:, sl], scalar=nmi_c[:],
                in1=lnbr_sb[:, sl], op0=MULT, op1=ADD)
            nc.vector.scalar_tensor_tensor(
                out=y_sb[:, sl], in0=hpre_sb[:, sl], scalar=istd_c[:],
                in1=lnsr_sb[:, sl], op0=MULT, op1=MULT)
            nc.vector.tensor_add(out_sb[:, sl], y_sb[:, sl], t2_sb[:, sl])
        else:
            # ln_scale==1, ln_bias==0: out = hpre*istd - m*istd
            if h == 0:
                nc.scalar.activation(
                    out_sb[:, sl], hpre_sb[:, sl],
                    mybir.ActivationFunctionType.Identity,
                    scale=istd_c[:], bias=nmi_c[:])
            else:
                nc.vector.tensor_scalar(
                    out=out_sb[:, sl], in0=hpre_sb[:, sl],
                    scalar1=istd_c[:], scalar2=nmi_c[:], op0=MULT, op1=ADD)
        eng = nc.sync if h == 0 else nc.scalar
        eng.dma_start(out=out_d.ap()[:, sl], in_=out_sb[:, sl])


def _dr_layout(m, scale):
    """[1024 k, 1024 out] f32 -> [128, (h j i n)] fp8 DoubleRow layout."""
    a = np.asarray(m * scale, dtype=NP_F8)
    # k -> (j, i, p), out -> (h, n); final [p, h, j, i, n]
    a = a.reshape(JT, 2, 128, NH, 512).transpose(2, 3, 0, 1, 4)
    return np.ascontiguousarray(a.reshape(128, KT * D))


def _prep_in_maps(inputs, general_ln):
    def f32c(x):
        return np.ascontiguousarray(np.asarray(x, dtype=np.float32))

    h_a = f32c(inputs["h_A"])
    alpha = f32c(inputs["alpha"])
    pool = np.asarray(inputs["pool_vectors"], dtype=np.float32)
    w_base = np.asarray(inputs["W_base"], dtype=np.float32)

    # pool_vectors rows: [U_n (D*R) | V_n (R*D) | bias_n (D)]
    u = pool[:, :D * R_RANK].reshape(N_EXP, D, R_RANK)
    v = pool[:, D * R_RANK:2 * D * R_RANK].reshape(N_EXP, R_RANK, D)
    bias_pool = pool[:, 2 * D * R_RANK:]                    # [64, D]
    bb = np.asarray(inputs["b_base"], dtype=np.float32).reshape(1, D)
    # fp8 weights are scaled x32; alpha carries 1/32, so the bias rows
    # need x(32*32) for the pool part and x32 for b_base
    bp = np.concatenate([bias_pool * (4 * WSC), bb * (4 * WSC)], axis=0)
    vt = _dr_layout(v.reshape(N_EXP * R_RANK, D).T, WSC)   # [a, (n,r)]
    wt = _dr_layout(w_base.T, WSC)                          # [a, c]
    ut = _dr_layout(u.transpose(0, 2, 1).reshape(N_EXP * R_RANK, D), WSC)
    gm = float(np.asarray(inputs["gamma"], dtype=np.float32)) / WSC

    in_maps = []
    for k in range(N_CORES):
        rows = slice(B_CORE * k, B_CORE * (k + 1))
        xt = h_a[rows].T                                    # [1024, 32]
        x8 = np.asarray(xt.reshape(JT, 2, 128, B_CORE).transpose(2, 0, 1, 3)
                        .reshape(128, XW), dtype=NP_F8)
        fb = np.concatenate(
            [h_a[rows], np.full((B_CORE, 1), gm, np.float32)], axis=1)
        alt = np.concatenate(
            [alpha[rows] / 4.0, np.full((B_CORE, 1), 0.25, np.float32)],
            axis=1).T
        bbb = np.concatenate([bp, alt], axis=1)             # [65, 1056]
        bb8 = np.zeros((128, D + B_CORE), np.float32)
        bb8[:N_EXP + 1] = bbb
        al8 = np.zeros((128, N_EXP), np.float32)
        al8[:B_CORE] = alpha[rows] / 4.0
        xvb = np.concatenate(
            [x8, np.asarray(bb8, dtype=NP_F8), np.asarray(al8, dtype=NP_F8),
             vt], axis=1)
        im = {
            "xv": np.ascontiguousarray(xvb),
            "fb": f32c(fb), "wt": wt, "ut": ut,
        }
        if general_ln:
            im["lns"] = f32c(inputs["ln_scale"]).reshape(1, D)
            im["lnb"] = f32c(inputs["ln_bias"]).reshape(1, D)
        in_maps.append(im)
    return in_maps


def _is_general_ln(inputs):
    lns = np.asarray(inputs["ln_scale"], dtype=np.float32)
    lnb = np.asarray(inputs["ln_bias"], dtype=np.float32)
    return not (np.all(lns == 1.0) and np.all(lnb == 0.0))


def get_compiled(general_ln=False):
    key = bool(general_ln)
    if key not in _COMPILED:
        _COMPILED[key] = _build(key)
    return _COMPILED[key]


def kernel(**inputs):
    general_ln = _is_general_ln(inputs)
    nc = get_compiled(general_ln)
    in_maps = _prep_in_maps(inputs, general_ln)
    res = bass_utils.run_bass_kernel_spmd(
        nc, in_maps, core_ids=list(range(N_CORES)))
    return np.concatenate([r["out"] for r in res.results], axis=0)
